# revision 19
# baseline (speedup 1.0000x reference)
"""Trainium2 Bass kernel for nn_BatchedPrecisionAttentionBlock.

Math (reference semantics):
  complex QKV projections; pairwise propagation kernel Kexp(l) and Kalman
  precision P(l) depend only on lag l = i-j (uniform time grid) -> small
  zero-padded lag tables instead of (S,S,D) transcendentals; zero padding
  for l<0 doubles as the causal mask.

Device computes, per (i-block 32 x j-chunk 64) work item:
  Xhat = Kexp (x) V                      (complex hadamard, stacked re/im)
  R    = Q - Kexp (x) (C*K)              (residual)
  maha = sum_d NU*P*R^2  (TensorE ones-matmul, PSUM pre-seeded with 1.0)
  A    = P / (1 + maha)  (reciprocal-approx + mult)
Host does: QKV projections, lag tables, row normalization (Qij = A/rowsum),
est_v/est/pred/out projections, gather + layout transpose.

Sharding: 8 cores = 2 batches x 4 paired row groups; core m of a batch owns
rows [64m,64m+64) u [448-64m,512-64m)  -> every core has exactly 18 equal
work items (perfect triangular balance) -> one uniform SPMD program; all
per-core differences live in host-packed input data.
"""

import numpy as np
from contextlib import ExitStack

import concourse.bass as bass
import concourse.bacc as bacc
import concourse.tile as tile
from concourse import mybir
from concourse.bass_utils import run_bass_kernel_spmd

# problem constants (hardcoded per harness contract)
B, S, H, D = 2, 512, 64, 64
DT = 0.01
NU, DELTA, ETA = 0.5, 1.0, 1.0

BI, BJ = 32, 64          # work item = BI query rows x BJ key cols
FREE = BI * BJ           # 2048
TABW = BI + BJ           # 96 table slots per item (slot 95 unused pad)
NITEMS = 18              # items per core (constant across cores by pairing)
NCORES = 8

# offsets of the sub-arrays inside the single packed input (per-partition)
O_KRR = 0
O_KIN = O_KRR + NITEMS * TABW
O_P = O_KIN + NITEMS * TABW
O_PN = O_P + NITEMS * TABW
O_V2 = O_PN + NITEMS * TABW
O_VSW = O_V2 + NITEMS * BJ
O_KK2 = O_VSW + NITEMS * BJ
O_KKSW = O_KK2 + NITEMS * BJ
O_QP = O_KKSW + NITEMS * BJ
PACK_TOT = O_QP + NITEMS * BI

F32 = mybir.dt.float32


def build_items(m: int):
    """Work items (i0, j0) for row-pair-group m of a batch."""
    blocks = [64 * m, 64 * m + 32, 448 - 64 * m, 480 - 64 * m]
    items = []
    for i0 in blocks:
        nch = (i0 + BI + BJ - 1) // BJ
        for jc in range(nch):
            items.append((i0, jc * BJ))
    assert len(items) == NITEMS, (m, len(items))
    return items


# ---------------------------------------------------------------- device ---

_NC_CACHE = {}


def build_nc():
    if "nc" in _NC_CACHE:
        return _NC_CACHE["nc"]
    nc = bacc.Bacc("TRN2")

    packall = nc.declare_dram_parameter("packall", [128, PACK_TOT], F32, isOutput=False)

    xout = nc.declare_dram_parameter("xout", [128, NITEMS * FREE], F32, isOutput=True)
    aout = nc.declare_dram_parameter("aout", [64, NITEMS * FREE], F32, isOutput=True)

    def tab_ap(t, base):
        a = t[:]
        return bass.AP(a.tensor, a.offset + base, [list(a.ap[0]), [1, BI], [-1, BJ]])

    def bc_j(t, base):
        a = t[:]
        return bass.AP(a.tensor, a.offset + base, [list(a.ap[0]), [0, BI], [1, BJ]])

    def bc_i(t, base):
        a = t[:]
        return bass.AP(a.tensor, a.offset + base, [list(a.ap[0]), [1, BI], [0, BJ]])

    def shp(t, p0=0, pn=128):
        a = t[p0:pn]
        return bass.AP(a.tensor, a.offset, [list(a.ap[0]), [BJ, BI], [1, BJ]])

    with tile.TileContext(nc) as tc, ExitStack() as ctx:
        const = ctx.enter_context(tc.tile_pool(name="const", bufs=1))
        big = ctx.enter_context(tc.tile_pool(name="big", bufs=3))
        psum = ctx.enter_context(tc.tile_pool(name="psum", bufs=8, space="PSUM"))

        s_all = const.tile([128, PACK_TOT], F32)
        nc.sync.dma_start(out=s_all[:], in_=packall[:])

        ones_l = const.tile([128, 128], F32)
        nc.vector.memset(ones_l[:], 1.0)
        ones_r = const.tile([1, 512], F32)
        nc.vector.memset(ones_r[:], 1.0)


        for w in range(NITEMS):
            tb = w * TABW + (BJ - 1)
            vb = w * BJ
            qb = w * BI

            t1 = big.tile([128, FREE], F32, tag="t1")
            t2 = big.tile([128, FREE], F32, tag="t2")
            k1 = big.tile([128, FREE], F32, tag="k1")
            k2 = big.tile([128, FREE], F32, tag="k2")

            # Xhat = [Kr;Kr]*[Vr;Vi] - [Ki;-Ki]*[Vi;Vr]
            nc.vector.tensor_mul(shp(t1), tab_ap(s_all, O_KRR + tb), bc_j(s_all, O_V2 + vb))
            nc.gpsimd.tensor_mul(shp(t2), tab_ap(s_all, O_KIN + tb), bc_j(s_all, O_VSW + vb))
            nc.vector.tensor_sub(shp(t1), shp(t1), shp(t2))
            nc.sync.dma_start(out=xout[:, w * FREE:(w + 1) * FREE], in_=t1[:])

            # R = [Qr;Qi] - ([Kr;Kr]*[Kkr;Kki] - [Ki;-Ki]*[Kki;Kkr])
            nc.vector.tensor_mul(shp(k1), tab_ap(s_all, O_KRR + tb), bc_j(s_all, O_KK2 + vb))
            nc.gpsimd.tensor_mul(shp(k2), tab_ap(s_all, O_KIN + tb), bc_j(s_all, O_KKSW + vb))
            nc.vector.tensor_sub(shp(k1), bc_i(s_all, O_QP + qb), shp(k1))
            nc.vector.tensor_add(shp(k2), shp(k1), shp(k2))
            # sq = R^2 (ACT), msq = NU*P * sq
            nc.scalar.activation(shp(k1), shp(k2), mybir.ActivationFunctionType.Square)
            nc.vector.tensor_mul(shp(k2), tab_ap(s_all, O_PN + tb), shp(k1))

            # denom = 1 + sum_d msq : PSUM seeded with ones then accumulated
            for c in range(FREE // 512):
                sl = slice(c * 512, (c + 1) * 512)
                ps = psum.tile([128, 512], F32, tag="ps")
                nc.tensor.matmul(ps[:], ones_l[0:1, :], ones_r[:],
                                 start=True, stop=False)
                nc.tensor.matmul(ps[:], ones_l[:], k2[:, sl],
                                 start=False, stop=True)
                # rd = 1/denom
                nc.vector.reciprocal_approx_fast(out=k1[:, sl], in_=ps[:])
            nc.vector.tensor_mul(shp(k2), tab_ap(s_all, O_P + tb), shp(k1))
            nc.sync.dma_start(out=aout[:, w * FREE:(w + 1) * FREE], in_=k2[0:64, :])

    nc.compile()   # bacc passes: split sync waits to HW limits, DCE, etc.
    _NC_CACHE["nc"] = nc
    return nc


# ------------------------------------------------------------------ host ---

def _cmm(W, b, X):
    """Complex matmul+bias. W:(2,O,I), b:(2,O,1), X:(B,2,S,I) -> (B,2,S,O)."""
    Yr = X[:, 0] @ W[0].T - X[:, 1] @ W[1].T + b[0].T
    Yi = X[:, 0] @ W[1].T + X[:, 1] @ W[0].T + b[1].T
    return np.stack([Yr, Yi], axis=1)


def _tables(lambda1, lam_Omega_sqrt, lam_Omega0_sqrt, lam_Gamma_sqrt, lam_C):
    lr = -np.abs(lambda1[0, :, 0].astype(np.float64))
    li = lambda1[1, :, 0].astype(np.float64)
    real = np.concatenate([lr, lr])
    imag = np.concatenate([li, -li])
    Om = (lam_Omega_sqrt.astype(np.float64) ** 2)[0, :, 0]
    Om0 = (lam_Omega0_sqrt.astype(np.float64) ** 2)[0, :, 0]
    Gam = (lam_Gamma_sqrt.astype(np.float64) ** 2)[0, :, 0]
    C = lam_C.astype(np.float64)[0, :, 0]

    l = np.arange(S, dtype=np.float64)
    dt = l * DT
    decay = np.exp(real[None, :] * dt[:, None])          # (S, D)
    ph = imag[None, :] * dt[:, None]
    Kr = decay * np.cos(ph)
    Ki = decay * np.sin(ph)
    exp2 = np.exp(2.0 * real[None, :] * dt[:, None])
    var = exp2 * Om0[None, :] + Om[None, :] * (exp2 - 1.0) / (2.0 * real[None, :])
    P = 1.0 / (C[None, :] ** 2 * var + Gam[None, :])     # (S, D)
    return real, imag, C, Kr, Ki, P


def _pack_tables(items, Kr, Ki, P):
    """(128, NITEMS, TABW) stacked lag tables, zero for out-of-range lags."""
    def pack(top, bot):
        out = np.zeros((128, NITEMS, TABW), np.float32)
        for w, (i0, j0) in enumerate(items):
            l0 = i0 - j0 - (BJ - 1)
            ls = np.arange(l0, l0 + TABW)
            valid = (ls >= 0) & (ls < S)
            lv = ls[valid]
            out[0:64, w, valid] = top[lv].T
            out[64:128, w, valid] = bot[lv].T
        return out.reshape(128, NITEMS * TABW)
    t_krr = pack(Kr, Kr)
    t_kin = pack(Ki, -Ki)
    t_p = pack(P, P)
    t_pn = pack(NU * P, NU * P)
    return t_krr, t_kin, t_p, t_pn


def prepare(inputs):
    """Host prep: projections, lag tables, per-core packed device inputs."""
    X_q = np.asarray(inputs["X_q"], np.float64)
    X_k = np.asarray(inputs["X_k"], np.float64)
    X_v = np.asarray(inputs["X_v"], np.float64)
    lambda1 = np.asarray(inputs["lambda1"])
    W = {k: np.asarray(inputs[k], np.float64)
         for k in ["W_q", "W_k", "W_v", "W_p", "W_e",
                   "b_q", "b_k", "b_v", "b_p", "b_e"]}

    real, imag, C, Kr, Ki, P = _tables(
        lambda1, inputs["lam_Omega_sqrt"], inputs["lam_Omega0_sqrt"],
        inputs["lam_Gamma_sqrt"], inputs["lam_C"])

    Q = _cmm(W["W_q"], W["b_q"], X_q)          # (B,2,S,D) float64
    Kk = _cmm(W["W_k"], W["b_k"], X_k) * C[None, None, None, :]
    V = _cmm(W["W_v"], W["b_v"], X_v)

    # ---- per-core input packs -------------------------------------------
    in_maps = []
    core_items = []
    for core in range(NCORES):
        b, m = core // 4, core % 4
        items = build_items(m)
        core_items.append((b, items))
        t_krr, t_kin, t_p, t_pn = _pack_tables(items, Kr, Ki, P)

        def packj(top, bot):
            out = np.empty((128, NITEMS, BJ), np.float32)
            for w, (_, j0) in enumerate(items):
                out[0:64, w] = top[:, j0:j0 + BJ]
                out[64:128, w] = bot[:, j0:j0 + BJ]
            return out.reshape(128, NITEMS * BJ)

        Vr, Vi = V[b, 0].T, V[b, 1].T            # (D, S)
        Kkr, Kki = Kk[b, 0].T, Kk[b, 1].T
        qpk = np.empty((128, NITEMS, BI), np.float32)
        for w, (i0, _) in enumerate(items):
            qpk[0:64, w] = Q[b, 0].T[:, i0:i0 + BI]
            qpk[64:128, w] = Q[b, 1].T[:, i0:i0 + BI]

        pk = np.empty((128, PACK_TOT), np.float32)
        pk[:, O_KRR:O_KIN] = t_krr
        pk[:, O_KIN:O_P] = t_kin
        pk[:, O_P:O_PN] = t_p
        pk[:, O_PN:O_V2] = t_pn
        pk[:, O_V2:O_VSW] = packj(Vr, Vi)
        pk[:, O_VSW:O_KK2] = packj(Vi, Vr)
        pk[:, O_KK2:O_KKSW] = packj(Kkr, Kki)
        pk[:, O_KKSW:O_QP] = packj(Kki, Kkr)
        pk[:, O_QP:PACK_TOT] = qpk.reshape(128, NITEMS * BI)
        in_maps.append({"packall": pk})

    return in_maps, core_items, (real, imag, W)


def kernel(**inputs):
    t_all = np.asarray(inputs["t_measure_all"], np.float64)
    in_maps, core_items, (real, imag, W) = prepare(inputs)

    nc = build_nc()
    global _last_in_maps
    _last_in_maps = in_maps
    res = run_bass_kernel_spmd(nc, in_maps, core_ids=list(range(NCORES)))

    # ---- gather / unshard ------------------------------------------------
    Xhat = np.zeros((B, 2, S, S, D), np.float32)
    A = np.zeros((B, S, S, D), np.float32)
    for core in range(NCORES):
        b, items = core_items[core]
        xo = res.results[core]["xout"].reshape(2, 64, NITEMS, BI, BJ)
        xo = np.ascontiguousarray(xo.transpose(2, 0, 3, 4, 1))  # (w,2,BI,BJ,D)
        ao = res.results[core]["aout"].reshape(64, NITEMS, BI, BJ)
        ao = np.ascontiguousarray(ao.transpose(1, 2, 3, 0))     # (w,BI,BJ,D)
        for w, (i0, j0) in enumerate(items):
            Xhat[b, :, i0:i0 + BI, j0:j0 + BJ, :] = xo[w]
            A[b, i0:i0 + BI, j0:j0 + BJ, :] = ao[w]

    rowsum = A.sum(axis=2, keepdims=True)       # (B,S,1,D)
    Qij = A
    Qij /= rowsum
    est_v = np.einsum("bijd,bcijd->bcid", Qij, Xhat).astype(np.float64)

    est = _cmm(W["W_e"], W["b_e"], est_v)
    dtl = t_all[:, -1] - t_all[:, -2]
    dr = np.exp(real[None, :] * dtl[:, None])
    pph = imag[None, :] * dtl[:, None]
    mr, mi = dr * np.cos(pph), dr * np.sin(pph)
    pr = mr[:, None] * est_v[:, 0] - mi[:, None] * est_v[:, 1]
    pi = mr[:, None] * est_v[:, 1] + mi[:, None] * est_v[:, 0]
    pred = _cmm(W["W_p"], W["b_p"], np.stack([pr, pi], axis=1))
    out = DELTA * pred + ETA * est

    lam = np.stack([real, imag]).astype(np.float32)
    return (est.astype(np.float32), out.astype(np.float32), Qij, Xhat, lam)


# revision 27
# speedup vs baseline: 1.3188x; 1.3188x over previous
"""Trainium2 Bass kernel for nn_BatchedPrecisionAttentionBlock.

Math (reference semantics):
  complex QKV projections; pairwise propagation kernel Kexp(l) and Kalman
  precision P(l) depend only on lag l = i-j (uniform time grid) -> small
  zero-padded lag tables instead of (S,S,D) transcendentals; zero padding
  for l<0 doubles as the causal mask.

Device computes, per (i-block 32 x j-chunk 64) work item:
  Xhat = Kexp (x) V                 fp32 (it is an output)
  R    = Q - Kexp (x) (C*K)         bf16 (feeds a 128-term fp32 PSUM sum)
  maha = sum_d NU*P*R^2             TensorE bf16 matmul w/ ones
  A    = P / (1 + maha)             fp32 (reciprocal-approx + mult)
Host does: QKV projections, lag tables, row normalization (Qij = A/rowsum),
est_v/est/pred/out projections, gather + layout transpose.

Sharding: 8 cores = 2 batches x 4 paired row groups; core m of a batch owns
rows [64m,64m+64) u [448-64m,512-64m) -> every core has exactly 18 equal
work items (perfect triangular balance) -> one uniform SPMD program; all
per-core differences live in host-packed input data.
"""

import numpy as np
import ml_dtypes
from contextlib import ExitStack

import concourse.bass as bass
import concourse.bacc as bacc
import concourse.tile as tile
from concourse import mybir
from concourse.bass_utils import run_bass_kernel_spmd

# problem constants (hardcoded per harness contract)
B, S, H, D = 2, 512, 64, 64
DT = 0.01
NU, DELTA, ETA = 0.5, 1.0, 1.0

BI, BJ = 32, 64          # work item = BI query rows x BJ key cols
FREE = BI * BJ           # 2048
TABW = BI + BJ           # 96 table slots per item (slot 95 unused pad)
NITEMS = 18              # items per core (constant across cores by pairing)
NCORES = 8

# per-item fp32 pack offsets (item-major so each item's inputs load separately)
O_KRR = 0
O_KIN = O_KRR + TABW
O_P = O_KIN + TABW
O_V2 = O_P + TABW
O_VSW = O_V2 + BJ
ITEM_F32 = O_VSW + BJ          # 416
# per-item bf16 pack offsets
H_KRR = 0
H_KIN = H_KRR + TABW
H_PN = H_KIN + TABW
H_KK2 = H_PN + TABW
H_KKSW = H_KK2 + BJ
H_Q = H_KKSW + BJ
ITEM_B16 = H_Q + BI            # 448

F32 = mybir.dt.float32
B16 = mybir.dt.bfloat16
BF16NP = ml_dtypes.bfloat16


def build_items(m: int):
    """Work items (i0, j0) for row-pair-group m of a batch."""
    blocks = [64 * m, 64 * m + 32, 448 - 64 * m, 480 - 64 * m]
    items = []
    for i0 in blocks:
        nch = (i0 + BI + BJ - 1) // BJ
        for jc in range(nch):
            items.append((i0, jc * BJ))
    assert len(items) == NITEMS, (m, len(items))
    return items


# ---------------------------------------------------------------- device ---

_NC_CACHE = {}


def build_nc():
    if "nc" in _NC_CACHE:
        return _NC_CACHE["nc"]
    nc = bacc.Bacc("TRN2")

    packf = nc.declare_dram_parameter("packf", [128, NITEMS * ITEM_F32], F32, isOutput=False)
    packh = nc.declare_dram_parameter("packh", [128, NITEMS * ITEM_B16], B16, isOutput=False)
    xout = nc.declare_dram_parameter("xout", [128, NITEMS * FREE], F32, isOutput=True)
    aout = nc.declare_dram_parameter("aout", [64, NITEMS * FREE], F32, isOutput=True)

    def tab_ap(t, base):          # Toeplitz table read: l = i-j (+offset)
        a = t[:]
        return bass.AP(a.tensor, a.offset + base, [list(a.ap[0]), [1, BI], [-1, BJ]])

    def bc_j(t, base):            # j-data broadcast over i
        a = t[:]
        return bass.AP(a.tensor, a.offset + base, [list(a.ap[0]), [0, BI], [1, BJ]])

    def bc_i(t, base):            # i-data broadcast over j
        a = t[:]
        return bass.AP(a.tensor, a.offset + base, [list(a.ap[0]), [1, BI], [0, BJ]])

    def shp(t):                   # (128, [BI, BJ]) view of a FREE tile
        a = t[:]
        return bass.AP(a.tensor, a.offset, [list(a.ap[0]), [BJ, BI], [1, BJ]])

    with tile.TileContext(nc) as tc, ExitStack() as ctx:
        const = ctx.enter_context(tc.tile_pool(name="const", bufs=1))
        big3 = ctx.enter_context(tc.tile_pool(name="big3", bufs=3))
        big2 = ctx.enter_context(tc.tile_pool(name="big2", bufs=2))
        psum = ctx.enter_context(tc.tile_pool(name="psum", bufs=8, space="PSUM"))

        ones_l = const.tile([128, 128], B16)
        nc.vector.memset(ones_l[:], 1.0)

        for w in range(NITEMS):
            tb = BJ - 1

            s_f = big3.tile([128, ITEM_F32], F32, tag="inf")
            nc.sync.dma_start(out=s_f[:], in_=packf[:, w * ITEM_F32:(w + 1) * ITEM_F32])
            s_h = big3.tile([128, ITEM_B16], B16, tag="inh")
            nc.sync.dma_start(out=s_h[:], in_=packh[:, w * ITEM_B16:(w + 1) * ITEM_B16])

            t1 = big3.tile([128, FREE], F32, tag="t1")
            t2 = big3.tile([128, FREE], F32, tag="t2")
            k1 = big3.tile([128, FREE], B16, tag="k1")
            k2 = big3.tile([128, FREE], B16, tag="k2")
            qx = big2.tile([128, FREE], B16, tag="qx")
            dnm = big2.tile([128, FREE], F32, tag="dnm")
            rdf = big2.tile([128, FREE], F32, tag="rdf")
            af = big2.tile([128, FREE], F32, tag="af")

            # Xhat = [Kr;Kr]*[Vr;Vi] - [Ki;-Ki]*[Vi;Vr]   (fp32)
            nc.vector.tensor_mul(shp(t1), tab_ap(s_f, O_KRR + tb), bc_j(s_f, O_V2))
            nc.gpsimd.tensor_mul(shp(t2), tab_ap(s_f, O_KIN + tb), bc_j(s_f, O_VSW))
            nc.gpsimd.tensor_sub(t1[:], t1[:], t2[:])
            nc.sync.dma_start(out=xout[:, w * FREE:(w + 1) * FREE], in_=t1[:])

            # R = [Qr;Qi] - ([Kr;Kr]*[Kkr;Kki] - [Ki;-Ki]*[Kki;Kkr])   (bf16)
            nc.scalar.activation(shp(qx), bc_i(s_h, H_Q),
                                 mybir.ActivationFunctionType.Copy)
            nc.vector.tensor_mul(shp(k1), tab_ap(s_h, H_KRR + tb), bc_j(s_h, H_KK2))
            nc.vector.tensor_mul(shp(k2), tab_ap(s_h, H_KIN + tb), bc_j(s_h, H_KKSW))
            nc.vector.tensor_sub(k2[:], k2[:], k1[:])
            nc.vector.tensor_add(k2[:], k2[:], qx[:])
            # sq = R^2 (ACT), msq = NU*P * sq  (bf16)
            nc.scalar.activation(k1[:], k2[:], mybir.ActivationFunctionType.Square)
            nc.vector.tensor_mul(shp(k2), tab_ap(s_h, H_PN + tb), shp(k1))

            # maha = sum_d msq (bf16 matmul w/ ones); denom = 1 + maha (ACT)
            for c in range(FREE // 512):
                sl = slice(c * 512, (c + 1) * 512)
                ps = psum.tile([128, 512], F32, tag="ps")
                nc.tensor.matmul(ps[:], ones_l[:], k2[:, sl], start=True, stop=True)
                nc.scalar.activation(dnm[:, sl], ps[:],
                                     mybir.ActivationFunctionType.Copy, bias=1.0)
            # rd = 1/denom ; A = P * rd   (fp32)
            nc.vector.reciprocal_approx_fast(out=rdf[:], in_=dnm[:])
            nc.vector.tensor_mul(shp(af), tab_ap(s_f, O_P + tb), shp(rdf))
            nc.sync.dma_start(out=aout[:, w * FREE:(w + 1) * FREE], in_=af[0:64, :])

    nc.compile()   # bacc passes: split sync waits to HW limits, DCE, etc.
    _NC_CACHE["nc"] = nc
    return nc


# ------------------------------------------------------------------ host ---

def _cmm(W, b, X):
    """Complex matmul+bias. W:(2,O,I), b:(2,O,1), X:(B,2,S,I) -> (B,2,S,O)."""
    Yr = X[:, 0] @ W[0].T - X[:, 1] @ W[1].T + b[0].T
    Yi = X[:, 0] @ W[1].T + X[:, 1] @ W[0].T + b[1].T
    return np.stack([Yr, Yi], axis=1)


def _tables(lambda1, lam_Omega_sqrt, lam_Omega0_sqrt, lam_Gamma_sqrt, lam_C):
    lr = -np.abs(lambda1[0, :, 0].astype(np.float64))
    li = lambda1[1, :, 0].astype(np.float64)
    real = np.concatenate([lr, lr])
    imag = np.concatenate([li, -li])
    Om = (lam_Omega_sqrt.astype(np.float64) ** 2)[0, :, 0]
    Om0 = (lam_Omega0_sqrt.astype(np.float64) ** 2)[0, :, 0]
    Gam = (lam_Gamma_sqrt.astype(np.float64) ** 2)[0, :, 0]
    C = lam_C.astype(np.float64)[0, :, 0]

    l = np.arange(S, dtype=np.float64)
    dt = l * DT
    decay = np.exp(real[None, :] * dt[:, None])          # (S, D)
    ph = imag[None, :] * dt[:, None]
    Kr = decay * np.cos(ph)
    Ki = decay * np.sin(ph)
    exp2 = np.exp(2.0 * real[None, :] * dt[:, None])
    var = exp2 * Om0[None, :] + Om[None, :] * (exp2 - 1.0) / (2.0 * real[None, :])
    P = 1.0 / (C[None, :] ** 2 * var + Gam[None, :])     # (S, D)
    return real, imag, C, Kr, Ki, P


def _pack_tab(items, top, bot, dtype):
    """(128, NITEMS, TABW) stacked lag table, zero for out-of-range lags."""
    out = np.zeros((128, NITEMS, TABW), dtype)
    for w, (i0, j0) in enumerate(items):
        l0 = i0 - j0 - (BJ - 1)
        ls = np.arange(l0, l0 + TABW)
        valid = (ls >= 0) & (ls < S)
        lv = ls[valid]
        out[0:64, w, valid] = top[lv].T.astype(dtype)
        out[64:128, w, valid] = bot[lv].T.astype(dtype)
    return out


def _pack_j(items, top, bot, dtype):
    out = np.empty((128, NITEMS, BJ), dtype)
    for w, (_, j0) in enumerate(items):
        out[0:64, w] = top[:, j0:j0 + BJ].astype(dtype)
        out[64:128, w] = bot[:, j0:j0 + BJ].astype(dtype)
    return out


def prepare(inputs):
    """Host prep: projections, lag tables, per-core packed device inputs."""
    X_q = np.asarray(inputs["X_q"], np.float64)
    X_k = np.asarray(inputs["X_k"], np.float64)
    X_v = np.asarray(inputs["X_v"], np.float64)
    lambda1 = np.asarray(inputs["lambda1"])
    W = {k: np.asarray(inputs[k], np.float64)
         for k in ["W_q", "W_k", "W_v", "W_p", "W_e",
                   "b_q", "b_k", "b_v", "b_p", "b_e"]}

    real, imag, C, Kr, Ki, P = _tables(
        lambda1, inputs["lam_Omega_sqrt"], inputs["lam_Omega0_sqrt"],
        inputs["lam_Gamma_sqrt"], inputs["lam_C"])

    Q = _cmm(W["W_q"], W["b_q"], X_q)          # (B,2,S,D) float64
    Kk = _cmm(W["W_k"], W["b_k"], X_k) * C[None, None, None, :]
    V = _cmm(W["W_v"], W["b_v"], X_v)

    in_maps = []
    core_items = []
    for core in range(NCORES):
        b, m = core // 4, core % 4
        items = build_items(m)
        core_items.append((b, items))

        Vr, Vi = V[b, 0].T, V[b, 1].T            # (D, S)
        Kkr, Kki = Kk[b, 0].T, Kk[b, 1].T

        pf = np.empty((128, NITEMS, ITEM_F32), np.float32)
        pf[:, :, O_KRR:O_KRR + TABW] = _pack_tab(items, Kr, Kr, np.float32)
        pf[:, :, O_KIN:O_KIN + TABW] = _pack_tab(items, Ki, -Ki, np.float32)
        pf[:, :, O_P:O_P + TABW] = _pack_tab(items, P, P, np.float32)
        pf[:, :, O_V2:O_V2 + BJ] = _pack_j(items, Vr, Vi, np.float32)
        pf[:, :, O_VSW:O_VSW + BJ] = _pack_j(items, Vi, Vr, np.float32)

        ph = np.empty((128, NITEMS, ITEM_B16), BF16NP)
        ph[:, :, H_KRR:H_KRR + TABW] = _pack_tab(items, Kr, Kr, BF16NP)
        ph[:, :, H_KIN:H_KIN + TABW] = _pack_tab(items, Ki, -Ki, BF16NP)
        ph[:, :, H_PN:H_PN + TABW] = _pack_tab(items, NU * P, NU * P, BF16NP)
        ph[:, :, H_KK2:H_KK2 + BJ] = _pack_j(items, Kkr, Kki, BF16NP)
        ph[:, :, H_KKSW:H_KKSW + BJ] = _pack_j(items, Kki, Kkr, BF16NP)
        qpk = np.empty((128, NITEMS, BI), BF16NP)
        for w, (i0, _) in enumerate(items):
            qpk[0:64, w] = Q[b, 0].T[:, i0:i0 + BI].astype(BF16NP)
            qpk[64:128, w] = Q[b, 1].T[:, i0:i0 + BI].astype(BF16NP)
        ph[:, :, H_Q:H_Q + BI] = qpk

        in_maps.append({"packf": pf.reshape(128, NITEMS * ITEM_F32),
                        "packh": ph.reshape(128, NITEMS * ITEM_B16)})

    return in_maps, core_items, (real, imag, W)


def kernel(**inputs):
    t_all = np.asarray(inputs["t_measure_all"], np.float64)
    in_maps, core_items, (real, imag, W) = prepare(inputs)

    nc = build_nc()
    global _last_in_maps
    _last_in_maps = in_maps
    res = run_bass_kernel_spmd(nc, in_maps, core_ids=list(range(NCORES)))

    # ---- gather / unshard ------------------------------------------------
    Xhat = np.zeros((B, 2, S, S, D), np.float32)
    A = np.zeros((B, S, S, D), np.float32)
    for core in range(NCORES):
        b, items = core_items[core]
        xo = res.results[core]["xout"].reshape(2, 64, NITEMS, BI, BJ)
        xo = np.ascontiguousarray(xo.transpose(2, 0, 3, 4, 1))  # (w,2,BI,BJ,D)
        ao = res.results[core]["aout"].reshape(64, NITEMS, BI, BJ)
        ao = np.ascontiguousarray(ao.transpose(1, 2, 3, 0))     # (w,BI,BJ,D)
        for w, (i0, j0) in enumerate(items):
            Xhat[b, :, i0:i0 + BI, j0:j0 + BJ, :] = xo[w]
            A[b, i0:i0 + BI, j0:j0 + BJ, :] = ao[w]

    rowsum = A.sum(axis=2, keepdims=True)       # (B,S,1,D)
    Qij = A
    Qij /= rowsum
    est_v = np.einsum("bijd,bcijd->bcid", Qij, Xhat).astype(np.float64)

    est = _cmm(W["W_e"], W["b_e"], est_v)
    dtl = t_all[:, -1] - t_all[:, -2]
    dr = np.exp(real[None, :] * dtl[:, None])
    pph = imag[None, :] * dtl[:, None]
    mr, mi = dr * np.cos(pph), dr * np.sin(pph)
    pr = mr[:, None] * est_v[:, 0] - mi[:, None] * est_v[:, 1]
    pi = mr[:, None] * est_v[:, 1] + mi[:, None] * est_v[:, 0]
    pred = _cmm(W["W_p"], W["b_p"], np.stack([pr, pi], axis=1))
    out = DELTA * pred + ETA * est

    lam = np.stack([real, imag]).astype(np.float32)
    return (est.astype(np.float32), out.astype(np.float32), Qij, Xhat, lam)


# revision 32
# speedup vs baseline: 1.4623x; 1.1088x over previous
"""Trainium2 Bass kernel for nn_BatchedPrecisionAttentionBlock.

Math (reference semantics):
  complex QKV projections; pairwise propagation kernel Kexp(l) and Kalman
  precision P(l) depend only on lag l = i-j (uniform time grid) -> small
  zero-padded lag tables instead of (S,S,D) transcendentals; zero padding
  for l<0 doubles as the causal mask.

Device computes, per (i-block 32 x j-chunk 64) work item:
  Xhat = Kexp (x) V                 fp32 (it is an output)
  R    = Q - Kexp (x) (C*K)         bf16 (feeds a 128-term fp32 PSUM sum)
  maha = sum_d NU*P*R^2             TensorE bf16 matmul w/ ones
  A    = P / (1 + maha)             fp32 (reciprocal-approx + mult)
Host does: QKV projections, lag tables, row normalization (Qij = A/rowsum),
est_v/est/pred/out projections, gather + layout transpose.

Sharding: 8 cores = 2 batches x 4 paired row groups; core m of a batch owns
rows [64m,64m+64) u [448-64m,512-64m) -> every core has exactly 18 equal
work items (perfect triangular balance) -> one uniform SPMD program; all
per-core differences live in host-packed input data.
"""

import numpy as np
import ml_dtypes
from contextlib import ExitStack

import concourse.bass as bass
import concourse.bacc as bacc
import concourse.tile as tile
from concourse import mybir
from concourse.bass_utils import run_bass_kernel_spmd

# problem constants (hardcoded per harness contract)
B, S, H, D = 2, 512, 64, 64
DT = 0.01
NU, DELTA, ETA = 0.5, 1.0, 1.0

BI, BJ = 32, 64          # work item = BI query rows x BJ key cols
FREE = BI * BJ           # 2048
TABW = BI + BJ           # 96 table slots per item (slot 95 unused pad)
NITEMS = 18              # items per core (constant across cores by pairing)
NCORES = 8

XHAT_BF16 = True               # bf16 Xhat pipeline (fp32 fallback if False)

# per-item fp32 pack offsets (item-major so each item's inputs load separately)
O_P = 0
if XHAT_BF16:
    ITEM_F32 = O_P + TABW      # 96: only the precision table stays fp32
    O_KRR = O_KIN = O_V2 = O_VSW = None
else:
    O_KRR = O_P + TABW
    O_KIN = O_KRR + TABW
    O_V2 = O_KIN + TABW
    O_VSW = O_V2 + BJ
    ITEM_F32 = O_VSW + BJ      # 416
# per-item bf16 pack offsets
H_KRR = 0
H_KIN = H_KRR + TABW
H_PN = H_KIN + TABW
H_KK2 = H_PN + TABW
H_KKSW = H_KK2 + BJ
H_Q = H_KKSW + BJ
if XHAT_BF16:
    H_V2 = H_Q + BI
    H_VSW = H_V2 + BJ
    ITEM_B16 = H_VSW + BJ      # 576
else:
    ITEM_B16 = H_Q + BI        # 448

F32 = mybir.dt.float32
B16 = mybir.dt.bfloat16
BF16NP = ml_dtypes.bfloat16


def build_items(m: int):
    """Work items (i0, j0) for row-pair-group m of a batch."""
    blocks = [64 * m, 64 * m + 32, 448 - 64 * m, 480 - 64 * m]
    items = []
    for i0 in blocks:
        nch = (i0 + BI + BJ - 1) // BJ
        for jc in range(nch):
            items.append((i0, jc * BJ))
    assert len(items) == NITEMS, (m, len(items))
    return items


# ---------------------------------------------------------------- device ---

_NC_CACHE = {}


def build_nc():
    if "nc" in _NC_CACHE:
        return _NC_CACHE["nc"]
    nc = bacc.Bacc("TRN2")

    XDT = B16 if XHAT_BF16 else F32
    packf = nc.declare_dram_parameter("packf", [128, NITEMS * ITEM_F32], F32, isOutput=False)
    packh = nc.declare_dram_parameter("packh", [128, NITEMS * ITEM_B16], B16, isOutput=False)
    xout = nc.declare_dram_parameter("xout", [128, NITEMS * FREE], XDT, isOutput=True)
    aout = nc.declare_dram_parameter("aout", [64, NITEMS * FREE], F32, isOutput=True)

    def tab_ap(t, base):          # Toeplitz table read: l = i-j (+offset)
        a = t[:]
        return bass.AP(a.tensor, a.offset + base, [list(a.ap[0]), [1, BI], [-1, BJ]])

    def bc_j(t, base):            # j-data broadcast over i
        a = t[:]
        return bass.AP(a.tensor, a.offset + base, [list(a.ap[0]), [0, BI], [1, BJ]])

    def bc_i(t, base):            # i-data broadcast over j
        a = t[:]
        return bass.AP(a.tensor, a.offset + base, [list(a.ap[0]), [1, BI], [0, BJ]])

    def shp(t):                   # (128, [BI, BJ]) view of a FREE tile
        a = t[:]
        return bass.AP(a.tensor, a.offset, [list(a.ap[0]), [BJ, BI], [1, BJ]])

    with tile.TileContext(nc) as tc, ExitStack() as ctx:
        const = ctx.enter_context(tc.tile_pool(name="const", bufs=1))
        big3 = ctx.enter_context(tc.tile_pool(name="big3", bufs=3))
        big2 = ctx.enter_context(tc.tile_pool(name="big2", bufs=2))
        psum = ctx.enter_context(tc.tile_pool(name="psum", bufs=8, space="PSUM"))

        ones_l = const.tile([128, 128], B16)
        nc.vector.memset(ones_l[:], 1.0)

        for w in range(NITEMS):
            tb = BJ - 1

            s_f = big3.tile([128, ITEM_F32], F32, tag="inf")
            nc.sync.dma_start(out=s_f[:], in_=packf[:, w * ITEM_F32:(w + 1) * ITEM_F32])
            s_h = big3.tile([128, ITEM_B16], B16, tag="inh")
            nc.sync.dma_start(out=s_h[:], in_=packh[:, w * ITEM_B16:(w + 1) * ITEM_B16])

            t1 = big3.tile([128, FREE], XDT, tag="t1")
            t2 = big3.tile([128, FREE], XDT, tag="t2")
            k1 = big3.tile([128, FREE], B16, tag="k1")
            k2 = big3.tile([128, FREE], B16, tag="k2")
            qx = big2.tile([128, FREE], B16, tag="qx")
            dnm = big2.tile([128, FREE], F32, tag="dnm")
            rdf = big2.tile([128, FREE], F32, tag="rdf")
            af = big2.tile([128, FREE], F32, tag="af")

            # Xhat = [Kr;Kr]*[Vr;Vi] - [Ki;-Ki]*[Vi;Vr]
            if XHAT_BF16:
                nc.vector.tensor_mul(shp(t1), tab_ap(s_h, H_KRR + tb), bc_j(s_h, H_V2))
                nc.gpsimd.tensor_mul(shp(t2), tab_ap(s_h, H_KIN + tb), bc_j(s_h, H_VSW))
            else:
                nc.vector.tensor_mul(shp(t1), tab_ap(s_f, O_KRR + tb), bc_j(s_f, O_V2))
                nc.gpsimd.tensor_mul(shp(t2), tab_ap(s_f, O_KIN + tb), bc_j(s_f, O_VSW))
            nc.gpsimd.tensor_sub(t1[:], t1[:], t2[:])
            nc.sync.dma_start(out=xout[:, w * FREE:(w + 1) * FREE], in_=t1[:])

            # R = [Qr;Qi] - ([Kr;Kr]*[Kkr;Kki] - [Ki;-Ki]*[Kki;Kkr])   (bf16)
            nc.scalar.activation(shp(qx), bc_i(s_h, H_Q),
                                 mybir.ActivationFunctionType.Copy)
            nc.vector.tensor_mul(shp(k1), tab_ap(s_h, H_KRR + tb), bc_j(s_h, H_KK2))
            nc.vector.tensor_mul(shp(k2), tab_ap(s_h, H_KIN + tb), bc_j(s_h, H_KKSW))
            nc.vector.tensor_sub(k2[:], k2[:], k1[:])
            nc.vector.tensor_add(k2[:], k2[:], qx[:])
            # sq = R^2 (ACT), msq = NU*P * sq  (bf16)
            nc.scalar.activation(k1[:], k2[:], mybir.ActivationFunctionType.Square)
            nc.vector.tensor_mul(shp(k2), tab_ap(s_h, H_PN + tb), shp(k1))

            # maha = sum_d msq (bf16 matmul w/ ones); denom = 1 + maha (ACT)
            for c in range(FREE // 512):
                sl = slice(c * 512, (c + 1) * 512)
                ps = psum.tile([128, 512], F32, tag="ps")
                nc.tensor.matmul(ps[:], ones_l[:], k2[:, sl], start=True, stop=True)
                nc.scalar.activation(dnm[:, sl], ps[:],
                                     mybir.ActivationFunctionType.Copy, bias=1.0)
            # rd = 1/denom ; A = P * rd   (fp32)
            nc.vector.reciprocal_approx_fast(out=rdf[:], in_=dnm[:])
            nc.vector.tensor_mul(shp(af), tab_ap(s_f, O_P + tb), shp(rdf))
            nc.sync.dma_start(out=aout[:, w * FREE:(w + 1) * FREE], in_=af[0:64, :])

    nc.compile()   # bacc passes: split sync waits to HW limits, DCE, etc.
    _NC_CACHE["nc"] = nc
    return nc


# ------------------------------------------------------------------ host ---

def _cmm(W, b, X):
    """Complex matmul+bias. W:(2,O,I), b:(2,O,1), X:(B,2,S,I) -> (B,2,S,O)."""
    Yr = X[:, 0] @ W[0].T - X[:, 1] @ W[1].T + b[0].T
    Yi = X[:, 0] @ W[1].T + X[:, 1] @ W[0].T + b[1].T
    return np.stack([Yr, Yi], axis=1)


def _tables(lambda1, lam_Omega_sqrt, lam_Omega0_sqrt, lam_Gamma_sqrt, lam_C):
    lr = -np.abs(lambda1[0, :, 0].astype(np.float64))
    li = lambda1[1, :, 0].astype(np.float64)
    real = np.concatenate([lr, lr])
    imag = np.concatenate([li, -li])
    Om = (lam_Omega_sqrt.astype(np.float64) ** 2)[0, :, 0]
    Om0 = (lam_Omega0_sqrt.astype(np.float64) ** 2)[0, :, 0]
    Gam = (lam_Gamma_sqrt.astype(np.float64) ** 2)[0, :, 0]
    C = lam_C.astype(np.float64)[0, :, 0]

    l = np.arange(S, dtype=np.float64)
    dt = l * DT
    decay = np.exp(real[None, :] * dt[:, None])          # (S, D)
    ph = imag[None, :] * dt[:, None]
    Kr = decay * np.cos(ph)
    Ki = decay * np.sin(ph)
    exp2 = np.exp(2.0 * real[None, :] * dt[:, None])
    var = exp2 * Om0[None, :] + Om[None, :] * (exp2 - 1.0) / (2.0 * real[None, :])
    P = 1.0 / (C[None, :] ** 2 * var + Gam[None, :])     # (S, D)
    return real, imag, C, Kr, Ki, P


def _pack_tab(items, top, bot, dtype):
    """(128, NITEMS, TABW) stacked lag table, zero for out-of-range lags."""
    out = np.zeros((128, NITEMS, TABW), dtype)
    for w, (i0, j0) in enumerate(items):
        l0 = i0 - j0 - (BJ - 1)
        ls = np.arange(l0, l0 + TABW)
        valid = (ls >= 0) & (ls < S)
        lv = ls[valid]
        out[0:64, w, valid] = top[lv].T.astype(dtype)
        out[64:128, w, valid] = bot[lv].T.astype(dtype)
    return out


def _pack_j(items, top, bot, dtype):
    out = np.empty((128, NITEMS, BJ), dtype)
    for w, (_, j0) in enumerate(items):
        out[0:64, w] = top[:, j0:j0 + BJ].astype(dtype)
        out[64:128, w] = bot[:, j0:j0 + BJ].astype(dtype)
    return out


def prepare(inputs):
    """Host prep: projections, lag tables, per-core packed device inputs."""
    X_q = np.asarray(inputs["X_q"], np.float64)
    X_k = np.asarray(inputs["X_k"], np.float64)
    X_v = np.asarray(inputs["X_v"], np.float64)
    lambda1 = np.asarray(inputs["lambda1"])
    W = {k: np.asarray(inputs[k], np.float64)
         for k in ["W_q", "W_k", "W_v", "W_p", "W_e",
                   "b_q", "b_k", "b_v", "b_p", "b_e"]}

    real, imag, C, Kr, Ki, P = _tables(
        lambda1, inputs["lam_Omega_sqrt"], inputs["lam_Omega0_sqrt"],
        inputs["lam_Gamma_sqrt"], inputs["lam_C"])

    Q = _cmm(W["W_q"], W["b_q"], X_q)          # (B,2,S,D) float64
    Kk = _cmm(W["W_k"], W["b_k"], X_k) * C[None, None, None, :]
    V = _cmm(W["W_v"], W["b_v"], X_v)

    in_maps = []
    core_items = []
    for core in range(NCORES):
        b, m = core // 4, core % 4
        items = build_items(m)
        core_items.append((b, items))

        Vr, Vi = V[b, 0].T, V[b, 1].T            # (D, S)
        Kkr, Kki = Kk[b, 0].T, Kk[b, 1].T

        pf = np.empty((128, NITEMS, ITEM_F32), np.float32)
        pf[:, :, O_P:O_P + TABW] = _pack_tab(items, P, P, np.float32)
        if not XHAT_BF16:
            pf[:, :, O_KRR:O_KRR + TABW] = _pack_tab(items, Kr, Kr, np.float32)
            pf[:, :, O_KIN:O_KIN + TABW] = _pack_tab(items, Ki, -Ki, np.float32)
            pf[:, :, O_V2:O_V2 + BJ] = _pack_j(items, Vr, Vi, np.float32)
            pf[:, :, O_VSW:O_VSW + BJ] = _pack_j(items, Vi, Vr, np.float32)

        ph = np.empty((128, NITEMS, ITEM_B16), BF16NP)
        ph[:, :, H_KRR:H_KRR + TABW] = _pack_tab(items, Kr, Kr, BF16NP)
        ph[:, :, H_KIN:H_KIN + TABW] = _pack_tab(items, Ki, -Ki, BF16NP)
        ph[:, :, H_PN:H_PN + TABW] = _pack_tab(items, NU * P, NU * P, BF16NP)
        ph[:, :, H_KK2:H_KK2 + BJ] = _pack_j(items, Kkr, Kki, BF16NP)
        ph[:, :, H_KKSW:H_KKSW + BJ] = _pack_j(items, Kki, Kkr, BF16NP)
        qpk = np.empty((128, NITEMS, BI), BF16NP)
        for w, (i0, _) in enumerate(items):
            qpk[0:64, w] = Q[b, 0].T[:, i0:i0 + BI].astype(BF16NP)
            qpk[64:128, w] = Q[b, 1].T[:, i0:i0 + BI].astype(BF16NP)
        ph[:, :, H_Q:H_Q + BI] = qpk
        if XHAT_BF16:
            ph[:, :, H_V2:H_V2 + BJ] = _pack_j(items, Vr, Vi, BF16NP)
            ph[:, :, H_VSW:H_VSW + BJ] = _pack_j(items, Vi, Vr, BF16NP)

        in_maps.append({"packf": pf.reshape(128, NITEMS * ITEM_F32),
                        "packh": ph.reshape(128, NITEMS * ITEM_B16)})

    return in_maps, core_items, (real, imag, W)


def kernel(**inputs):
    t_all = np.asarray(inputs["t_measure_all"], np.float64)
    in_maps, core_items, (real, imag, W) = prepare(inputs)

    nc = build_nc()
    global _last_in_maps
    _last_in_maps = in_maps
    res = run_bass_kernel_spmd(nc, in_maps, core_ids=list(range(NCORES)))

    # ---- gather / unshard ------------------------------------------------
    Xhat = np.zeros((B, 2, S, S, D), np.float32)
    A = np.zeros((B, S, S, D), np.float32)
    for core in range(NCORES):
        b, items = core_items[core]
        xo = res.results[core]["xout"].astype(np.float32).reshape(2, 64, NITEMS, BI, BJ)
        xo = np.ascontiguousarray(xo.transpose(2, 0, 3, 4, 1))  # (w,2,BI,BJ,D)
        ao = res.results[core]["aout"].reshape(64, NITEMS, BI, BJ)
        ao = np.ascontiguousarray(ao.transpose(1, 2, 3, 0))     # (w,BI,BJ,D)
        for w, (i0, j0) in enumerate(items):
            Xhat[b, :, i0:i0 + BI, j0:j0 + BJ, :] = xo[w]
            A[b, i0:i0 + BI, j0:j0 + BJ, :] = ao[w]

    rowsum = A.sum(axis=2, keepdims=True)       # (B,S,1,D)
    Qij = A
    Qij /= rowsum
    est_v = np.einsum("bijd,bcijd->bcid", Qij, Xhat).astype(np.float64)

    est = _cmm(W["W_e"], W["b_e"], est_v)
    dtl = t_all[:, -1] - t_all[:, -2]
    dr = np.exp(real[None, :] * dtl[:, None])
    pph = imag[None, :] * dtl[:, None]
    mr, mi = dr * np.cos(pph), dr * np.sin(pph)
    pr = mr[:, None] * est_v[:, 0] - mi[:, None] * est_v[:, 1]
    pi = mr[:, None] * est_v[:, 1] + mi[:, None] * est_v[:, 0]
    pred = _cmm(W["W_p"], W["b_p"], np.stack([pr, pi], axis=1))
    out = DELTA * pred + ETA * est

    lam = np.stack([real, imag]).astype(np.float32)
    return (est.astype(np.float32), out.astype(np.float32), Qij, Xhat, lam)


# revision 33
# speedup vs baseline: 1.7961x; 1.2283x over previous
"""Trainium2 Bass kernel for nn_BatchedPrecisionAttentionBlock.

Math (reference semantics):
  complex QKV projections; pairwise propagation kernel Kexp(l) and Kalman
  precision P(l) depend only on lag l = i-j (uniform time grid) -> small
  zero-padded lag tables instead of (S,S,D) transcendentals; zero padding
  for l<0 doubles as the causal mask.

Device computes, per (i-block 32 x j-chunk 64) work item:
  Xhat = Kexp (x) V                 fp32 (it is an output)
  R    = Q - Kexp (x) (C*K)         bf16 (feeds a 128-term fp32 PSUM sum)
  maha = sum_d NU*P*R^2             TensorE bf16 matmul w/ ones
  A    = P / (1 + maha)             fp32 (reciprocal-approx + mult)
Host does: QKV projections, lag tables, row normalization (Qij = A/rowsum),
est_v/est/pred/out projections, gather + layout transpose.

Sharding: 8 cores = 2 batches x 4 paired row groups; core m of a batch owns
rows [64m,64m+64) u [448-64m,512-64m) -> every core has exactly 18 equal
work items (perfect triangular balance) -> one uniform SPMD program; all
per-core differences live in host-packed input data.
"""

import numpy as np
import ml_dtypes
from contextlib import ExitStack

import concourse.bass as bass
import concourse.bacc as bacc
import concourse.tile as tile
from concourse import mybir
from concourse.bass_utils import run_bass_kernel_spmd

# problem constants (hardcoded per harness contract)
B, S, H, D = 2, 512, 64, 64
DT = 0.01
NU, DELTA, ETA = 0.5, 1.0, 1.0

BI, BJ = 32, 64          # work item = BI query rows x BJ key cols
FREE = BI * BJ           # 2048
TABW = BI + BJ           # 96 table slots per item (slot 95 unused pad)
NITEMS = 18              # items per core (constant across cores by pairing)
NCORES = 8

XHAT_BF16 = True               # bf16 Xhat pipeline (fp32 fallback if False)

# per-item fp32 pack offsets (item-major so each item's inputs load separately)
O_P = 0
if XHAT_BF16:
    ITEM_F32 = O_P + TABW      # 96: only the precision table stays fp32
    O_KRR = O_KIN = O_V2 = O_VSW = None
else:
    O_KRR = O_P + TABW
    O_KIN = O_KRR + TABW
    O_V2 = O_KIN + TABW
    O_VSW = O_V2 + BJ
    ITEM_F32 = O_VSW + BJ      # 416
# per-item bf16 pack offsets
H_KRR = 0
H_KIN = H_KRR + TABW
H_PN = H_KIN + TABW
H_KK2 = H_PN + TABW
H_KKSW = H_KK2 + BJ
H_Q = H_KKSW + BJ
if XHAT_BF16:
    H_V2 = H_Q + BI
    H_VSW = H_V2 + BJ
    ITEM_B16 = H_VSW + BJ      # 576
else:
    ITEM_B16 = H_Q + BI        # 448

F32 = mybir.dt.float32
B16 = mybir.dt.bfloat16
BF16NP = ml_dtypes.bfloat16


def build_items(m: int):
    """Work items (i0, j0) for row-pair-group m of a batch."""
    blocks = [64 * m, 64 * m + 32, 448 - 64 * m, 480 - 64 * m]
    items = []
    for i0 in blocks:
        nch = (i0 + BI + BJ - 1) // BJ
        for jc in range(nch):
            items.append((i0, jc * BJ))
    assert len(items) == NITEMS, (m, len(items))
    return items


# ---------------------------------------------------------------- device ---

_NC_CACHE = {}


def build_nc():
    if "nc" in _NC_CACHE:
        return _NC_CACHE["nc"]
    nc = bacc.Bacc("TRN2")

    XDT = B16 if XHAT_BF16 else F32
    packf = nc.declare_dram_parameter("packf", [128, NITEMS * ITEM_F32], F32, isOutput=False)
    packh = nc.declare_dram_parameter("packh", [128, NITEMS * ITEM_B16], B16, isOutput=False)
    xout = nc.declare_dram_parameter("xout", [128, NITEMS * FREE], XDT, isOutput=True)
    aout = nc.declare_dram_parameter("aout", [64, NITEMS * FREE], F32, isOutput=True)

    def tab_ap(t, base):          # Toeplitz table read: l = i-j (+offset)
        a = t[:]
        return bass.AP(a.tensor, a.offset + base, [list(a.ap[0]), [1, BI], [-1, BJ]])

    def bc_j(t, base):            # j-data broadcast over i
        a = t[:]
        return bass.AP(a.tensor, a.offset + base, [list(a.ap[0]), [0, BI], [1, BJ]])

    def bc_i(t, base):            # i-data broadcast over j
        a = t[:]
        return bass.AP(a.tensor, a.offset + base, [list(a.ap[0]), [1, BI], [0, BJ]])

    def shp(t):                   # (128, [BI, BJ]) view of a FREE tile
        a = t[:]
        return bass.AP(a.tensor, a.offset, [list(a.ap[0]), [BJ, BI], [1, BJ]])

    with tile.TileContext(nc) as tc, ExitStack() as ctx:
        const = ctx.enter_context(tc.tile_pool(name="const", bufs=1))
        big3 = ctx.enter_context(tc.tile_pool(name="big3", bufs=3))
        big2 = ctx.enter_context(tc.tile_pool(name="big2", bufs=2))
        psum = ctx.enter_context(tc.tile_pool(name="psum", bufs=8, space="PSUM"))

        ones_l = const.tile([128, 128], B16)
        nc.vector.memset(ones_l[:], 1.0)

        for w in range(NITEMS):
            tb = BJ - 1

            s_f = big3.tile([128, ITEM_F32], F32, tag="inf")
            nc.sync.dma_start(out=s_f[:], in_=packf[:, w * ITEM_F32:(w + 1) * ITEM_F32])
            s_h = big3.tile([128, ITEM_B16], B16, tag="inh")
            nc.sync.dma_start(out=s_h[:], in_=packh[:, w * ITEM_B16:(w + 1) * ITEM_B16])

            t1 = big3.tile([128, FREE], XDT, tag="t1")
            t2 = big3.tile([128, FREE], XDT, tag="t2")
            k1 = big3.tile([128, FREE], B16, tag="k1")
            k2 = big3.tile([128, FREE], B16, tag="k2")
            qx = big2.tile([128, FREE], B16, tag="qx")
            dnm = big2.tile([128, FREE], F32, tag="dnm")
            rdf = big2.tile([128, FREE], F32, tag="rdf")
            af = big2.tile([128, FREE], F32, tag="af")

            # Xhat = [Kr;Kr]*[Vr;Vi] - [Ki;-Ki]*[Vi;Vr]
            if XHAT_BF16:
                nc.vector.tensor_mul(shp(t1), tab_ap(s_h, H_KRR + tb), bc_j(s_h, H_V2))
                nc.vector.tensor_mul(shp(t2), tab_ap(s_h, H_KIN + tb), bc_j(s_h, H_VSW))
            else:
                nc.vector.tensor_mul(shp(t1), tab_ap(s_f, O_KRR + tb), bc_j(s_f, O_V2))
                nc.vector.tensor_mul(shp(t2), tab_ap(s_f, O_KIN + tb), bc_j(s_f, O_VSW))
            nc.vector.tensor_sub(t1[:], t1[:], t2[:])
            nc.sync.dma_start(out=xout[:, w * FREE:(w + 1) * FREE], in_=t1[:])

            # R = [Qr;Qi] - ([Kr;Kr]*[Kkr;Kki] - [Ki;-Ki]*[Kki;Kkr])   (bf16)
            nc.scalar.activation(shp(qx), bc_i(s_h, H_Q),
                                 mybir.ActivationFunctionType.Copy)
            nc.vector.tensor_mul(shp(k1), tab_ap(s_h, H_KRR + tb), bc_j(s_h, H_KK2))
            nc.vector.tensor_mul(shp(k2), tab_ap(s_h, H_KIN + tb), bc_j(s_h, H_KKSW))
            nc.vector.tensor_sub(k2[:], k2[:], k1[:])
            nc.vector.tensor_add(k2[:], k2[:], qx[:])
            # sq = R^2 (ACT), msq = NU*P * sq  (bf16)
            nc.scalar.activation(k1[:], k2[:], mybir.ActivationFunctionType.Square)
            nc.vector.tensor_mul(shp(k2), tab_ap(s_h, H_PN + tb), shp(k1))

            # maha = sum_d msq (bf16 matmul w/ ones); denom = 1 + maha (ACT)
            for c in range(FREE // 512):
                sl = slice(c * 512, (c + 1) * 512)
                ps = psum.tile([128, 512], F32, tag="ps")
                nc.tensor.matmul(ps[:], ones_l[:], k2[:, sl], start=True, stop=True)
                nc.scalar.activation(dnm[:, sl], ps[:],
                                     mybir.ActivationFunctionType.Copy, bias=1.0)
            # rd = 1/denom ; A = P * rd   (fp32)
            nc.vector.reciprocal_approx_fast(out=rdf[:], in_=dnm[:])
            nc.vector.tensor_mul(shp(af), tab_ap(s_f, O_P + tb), shp(rdf))
            nc.sync.dma_start(out=aout[:, w * FREE:(w + 1) * FREE], in_=af[0:64, :])

    nc.compile()   # bacc passes: split sync waits to HW limits, DCE, etc.
    _NC_CACHE["nc"] = nc
    return nc


# ------------------------------------------------------------------ host ---

def _cmm(W, b, X):
    """Complex matmul+bias. W:(2,O,I), b:(2,O,1), X:(B,2,S,I) -> (B,2,S,O)."""
    Yr = X[:, 0] @ W[0].T - X[:, 1] @ W[1].T + b[0].T
    Yi = X[:, 0] @ W[1].T + X[:, 1] @ W[0].T + b[1].T
    return np.stack([Yr, Yi], axis=1)


def _tables(lambda1, lam_Omega_sqrt, lam_Omega0_sqrt, lam_Gamma_sqrt, lam_C):
    lr = -np.abs(lambda1[0, :, 0].astype(np.float64))
    li = lambda1[1, :, 0].astype(np.float64)
    real = np.concatenate([lr, lr])
    imag = np.concatenate([li, -li])
    Om = (lam_Omega_sqrt.astype(np.float64) ** 2)[0, :, 0]
    Om0 = (lam_Omega0_sqrt.astype(np.float64) ** 2)[0, :, 0]
    Gam = (lam_Gamma_sqrt.astype(np.float64) ** 2)[0, :, 0]
    C = lam_C.astype(np.float64)[0, :, 0]

    l = np.arange(S, dtype=np.float64)
    dt = l * DT
    decay = np.exp(real[None, :] * dt[:, None])          # (S, D)
    ph = imag[None, :] * dt[:, None]
    Kr = decay * np.cos(ph)
    Ki = decay * np.sin(ph)
    exp2 = np.exp(2.0 * real[None, :] * dt[:, None])
    var = exp2 * Om0[None, :] + Om[None, :] * (exp2 - 1.0) / (2.0 * real[None, :])
    P = 1.0 / (C[None, :] ** 2 * var + Gam[None, :])     # (S, D)
    return real, imag, C, Kr, Ki, P


def _pack_tab(items, top, bot, dtype):
    """(128, NITEMS, TABW) stacked lag table, zero for out-of-range lags."""
    out = np.zeros((128, NITEMS, TABW), dtype)
    for w, (i0, j0) in enumerate(items):
        l0 = i0 - j0 - (BJ - 1)
        ls = np.arange(l0, l0 + TABW)
        valid = (ls >= 0) & (ls < S)
        lv = ls[valid]
        out[0:64, w, valid] = top[lv].T.astype(dtype)
        out[64:128, w, valid] = bot[lv].T.astype(dtype)
    return out


def _pack_j(items, top, bot, dtype):
    out = np.empty((128, NITEMS, BJ), dtype)
    for w, (_, j0) in enumerate(items):
        out[0:64, w] = top[:, j0:j0 + BJ].astype(dtype)
        out[64:128, w] = bot[:, j0:j0 + BJ].astype(dtype)
    return out


def prepare(inputs):
    """Host prep: projections, lag tables, per-core packed device inputs."""
    X_q = np.asarray(inputs["X_q"], np.float64)
    X_k = np.asarray(inputs["X_k"], np.float64)
    X_v = np.asarray(inputs["X_v"], np.float64)
    lambda1 = np.asarray(inputs["lambda1"])
    W = {k: np.asarray(inputs[k], np.float64)
         for k in ["W_q", "W_k", "W_v", "W_p", "W_e",
                   "b_q", "b_k", "b_v", "b_p", "b_e"]}

    real, imag, C, Kr, Ki, P = _tables(
        lambda1, inputs["lam_Omega_sqrt"], inputs["lam_Omega0_sqrt"],
        inputs["lam_Gamma_sqrt"], inputs["lam_C"])

    Q = _cmm(W["W_q"], W["b_q"], X_q)          # (B,2,S,D) float64
    Kk = _cmm(W["W_k"], W["b_k"], X_k) * C[None, None, None, :]
    V = _cmm(W["W_v"], W["b_v"], X_v)

    in_maps = []
    core_items = []
    for core in range(NCORES):
        b, m = core // 4, core % 4
        items = build_items(m)
        core_items.append((b, items))

        Vr, Vi = V[b, 0].T, V[b, 1].T            # (D, S)
        Kkr, Kki = Kk[b, 0].T, Kk[b, 1].T

        pf = np.empty((128, NITEMS, ITEM_F32), np.float32)
        pf[:, :, O_P:O_P + TABW] = _pack_tab(items, P, P, np.float32)
        if not XHAT_BF16:
            pf[:, :, O_KRR:O_KRR + TABW] = _pack_tab(items, Kr, Kr, np.float32)
            pf[:, :, O_KIN:O_KIN + TABW] = _pack_tab(items, Ki, -Ki, np.float32)
            pf[:, :, O_V2:O_V2 + BJ] = _pack_j(items, Vr, Vi, np.float32)
            pf[:, :, O_VSW:O_VSW + BJ] = _pack_j(items, Vi, Vr, np.float32)

        ph = np.empty((128, NITEMS, ITEM_B16), BF16NP)
        ph[:, :, H_KRR:H_KRR + TABW] = _pack_tab(items, Kr, Kr, BF16NP)
        ph[:, :, H_KIN:H_KIN + TABW] = _pack_tab(items, Ki, -Ki, BF16NP)
        ph[:, :, H_PN:H_PN + TABW] = _pack_tab(items, NU * P, NU * P, BF16NP)
        ph[:, :, H_KK2:H_KK2 + BJ] = _pack_j(items, Kkr, Kki, BF16NP)
        ph[:, :, H_KKSW:H_KKSW + BJ] = _pack_j(items, Kki, Kkr, BF16NP)
        qpk = np.empty((128, NITEMS, BI), BF16NP)
        for w, (i0, _) in enumerate(items):
            qpk[0:64, w] = Q[b, 0].T[:, i0:i0 + BI].astype(BF16NP)
            qpk[64:128, w] = Q[b, 1].T[:, i0:i0 + BI].astype(BF16NP)
        ph[:, :, H_Q:H_Q + BI] = qpk
        if XHAT_BF16:
            ph[:, :, H_V2:H_V2 + BJ] = _pack_j(items, Vr, Vi, BF16NP)
            ph[:, :, H_VSW:H_VSW + BJ] = _pack_j(items, Vi, Vr, BF16NP)

        in_maps.append({"packf": pf.reshape(128, NITEMS * ITEM_F32),
                        "packh": ph.reshape(128, NITEMS * ITEM_B16)})

    return in_maps, core_items, (real, imag, W)


def kernel(**inputs):
    t_all = np.asarray(inputs["t_measure_all"], np.float64)
    in_maps, core_items, (real, imag, W) = prepare(inputs)

    nc = build_nc()
    global _last_in_maps
    _last_in_maps = in_maps
    res = run_bass_kernel_spmd(nc, in_maps, core_ids=list(range(NCORES)))

    # ---- gather / unshard ------------------------------------------------
    Xhat = np.zeros((B, 2, S, S, D), np.float32)
    A = np.zeros((B, S, S, D), np.float32)
    for core in range(NCORES):
        b, items = core_items[core]
        xo = res.results[core]["xout"].astype(np.float32).reshape(2, 64, NITEMS, BI, BJ)
        xo = np.ascontiguousarray(xo.transpose(2, 0, 3, 4, 1))  # (w,2,BI,BJ,D)
        ao = res.results[core]["aout"].reshape(64, NITEMS, BI, BJ)
        ao = np.ascontiguousarray(ao.transpose(1, 2, 3, 0))     # (w,BI,BJ,D)
        for w, (i0, j0) in enumerate(items):
            Xhat[b, :, i0:i0 + BI, j0:j0 + BJ, :] = xo[w]
            A[b, i0:i0 + BI, j0:j0 + BJ, :] = ao[w]

    rowsum = A.sum(axis=2, keepdims=True)       # (B,S,1,D)
    Qij = A
    Qij /= rowsum
    est_v = np.einsum("bijd,bcijd->bcid", Qij, Xhat).astype(np.float64)

    est = _cmm(W["W_e"], W["b_e"], est_v)
    dtl = t_all[:, -1] - t_all[:, -2]
    dr = np.exp(real[None, :] * dtl[:, None])
    pph = imag[None, :] * dtl[:, None]
    mr, mi = dr * np.cos(pph), dr * np.sin(pph)
    pr = mr[:, None] * est_v[:, 0] - mi[:, None] * est_v[:, 1]
    pi = mr[:, None] * est_v[:, 1] + mi[:, None] * est_v[:, 0]
    pred = _cmm(W["W_p"], W["b_p"], np.stack([pr, pi], axis=1))
    out = DELTA * pred + ETA * est

    lam = np.stack([real, imag]).astype(np.float32)
    return (est.astype(np.float32), out.astype(np.float32), Qij, Xhat, lam)
